# revision 1
# baseline (speedup 1.0000x reference)
"""Cross-attention layer (B=2, L=2048, D=1024, 16 heads) on 8 TRN2 NeuronCores.

Two-phase pipeline: phase 1 computes K^T / V projections sharded 8-way
over kv rows (no replication); host regathers per batch; phase 2 runs
Q-projection + attention + output projection + LayerNorm row-sharded.

Phase 1, core c (b = c//4, kv rows 512*(c%4)..):
    KT_part[hd, kv_slice] = (Wk^T kvT_slice) + bk,  V_part = kv_slice @ Wv
Phase 2, core c (b = c//4, q rows 512*(c%4)..): identical attention pipeline
to kernel.py but K^T / ones-augmented V arrive via DRAM instead of on-core
projection.
"""

import numpy as np

import concourse.mybir as mybir
import concourse.tile as tile
from concourse import bacc
from concourse.bass_utils import run_bass_kernel_spmd

dt = mybir.dt
AF = mybir.ActivationFunctionType
ALU = mybir.AluOpType

P = 128
B, LQ, LKV = 2, 2048, 2048
DQ, DKV, HID, NH = 1024, 1024, 1024, 16
HD = HID // NH
EPS = 1e-5
N_CORES = 8
RQ = LQ * B // N_CORES             # 512
RKV = LKV * B // N_CORES           # 512 kv rows per phase-1 core
KV_T = LKV // P                    # 16
DPO = DQ // P                      # 8
N_PAIR = NH // 2                   # 8
MQ = RQ // P                       # 4
VA = HD + 1                        # 65


def build_phase1():
    nc = bacc.Bacc("TRN2", target_bir_lowering=False, debug=False,
                   num_devices=N_CORES)
    f32r, f32 = dt.float32r, dt.float32
    kvTs_d = nc.dram_tensor("kvTs", [DKV, RKV], f32r, kind="ExternalInput")
    wk_d = nc.dram_tensor("wk", [DKV, HID], f32r, kind="ExternalInput")
    wv_d = nc.dram_tensor("wv", [DKV, HID], f32r, kind="ExternalInput")
    bk_d = nc.dram_tensor("bk", [P, DPO], f32, kind="ExternalInput")
    ktp_d = nc.dram_tensor("ktp", [HID, RKV], f32, kind="ExternalOutput")
    vp_d = nc.dram_tensor("vp", [RKV, HID], f32, kind="ExternalOutput")

    with tile.TileContext(nc) as tc:
        with (
            tc.tile_pool(name="c1", bufs=1) as c1,
            tc.tile_pool(name="wkp", bufs=8) as wkp,
            tc.tile_pool(name="wvp", bufs=3) as wvp,
            tc.tile_pool(name="op", bufs=5) as op,
            tc.tile_pool(name="ps", bufs=8, space="PSUM") as ps,
        ):
            kvTs = c1.tile([P, DPO, RKV], f32r)
            for po in range(DPO):
                nc.sync.dma_start(
                    kvTs[:, po],
                    kvTs_d.ap().rearrange("(po p) q -> po p q", p=P)[po])
            bk_all = c1.tile([P, DPO], f32)
            nc.sync.dma_start(bk_all[:], bk_d.ap())
            wk_r = wk_d.ap().rearrange("(po p) h -> p po h", p=P)
            wv_r = wv_d.ap().rearrange("(po p) h -> p po h", p=P)
            # prefetch all weight blocks up-front so the PE stream is dense
            wk_blks = []
            for hc in range(DPO):
                wkb = wkp.tile([P, DPO, P], f32r, tag="wk", name=f"wkb{hc}")
                nc.sync.dma_start(wkb[:], wk_r[:, :, P * hc:P * (hc + 1)])
                wk_blks.append(wkb)
            wv_blks = []
            for n in range(2):
                wvb = wvp.tile([P, DPO, 512], f32r, tag="wv", name=f"wvb{n}")
                nc.sync.dma_start(wvb[:], wv_r[:, :, 512 * n:512 * (n + 1)])
                wv_blks.append(wvb)

            # K^T po-outer: 8 parallel psum accumulators so the PE stream
            # is dense from the first kvTs chunk (keeps the p-state warm)
            ps_ks = [ps.tile([P, RKV], f32, tag="k", name=f"ps_k{_h}")
                     for _h in range(DPO)]
            for po in range(DPO):
                for hc in range(DPO):
                    nc.tensor.matmul(ps_ks[hc][:], wk_blks[hc][:, po],
                                     kvTs[:, po], start=(po == 0),
                                     stop=(po == DPO - 1))
            for hc in range(DPO):
                kt_o = op.tile([P, RKV], f32, tag="kt")
                nc.scalar.activation(kt_o[:], ps_ks[hc][:], AF.Identity,
                                     bias=bk_all[:, hc:hc + 1])
                nc.sync.dma_start(
                    ktp_d.ap().rearrange("(hc p) q -> hc p q", p=P)[hc], kt_o[:])

            # V: for each kv 128-chunk t, hd 512-chunk n
            for n in range(2):
                wv_blk = wv_blks[n]
                for t in range(RKV // P):
                    ps_v = ps.tile([P, RKV], f32, tag="k",
                                   name="ps_v")[:, :512]
                    for po in range(DPO):
                        nc.tensor.matmul(
                            ps_v[:], kvTs[:, po, P * t:P * (t + 1)],
                            wv_blk[:, po], start=(po == 0), stop=(po == DPO - 1))
                    v_o = op.tile([P, 512], f32, tag="v")
                    nc.vector.tensor_copy(v_o[:], ps_v[:])
                    nc.sync.dma_start(
                        vp_d.ap().rearrange("(t p) (n f) -> t n p f",
                                            p=P, f=512)[t, n], v_o[:])
    nc.compile()
    return nc


def build_phase2():
    nc = bacc.Bacc("TRN2", target_bir_lowering=False, debug=False,
                   num_devices=N_CORES)
    f32r, f32 = dt.float32r, dt.float32
    qT_d = nc.dram_tensor("qT", [DQ, RQ], f32r, kind="ExternalInput")
    kt_d = nc.dram_tensor("kt", [HID, LKV], f32r, kind="ExternalInput")
    va_d = nc.dram_tensor("va", [LKV, NH * VA], f32r, kind="ExternalInput")
    xq_d = nc.dram_tensor("xq", [RQ, HID], f32, kind="ExternalInput")
    wq_d = nc.dram_tensor("wq", [DQ, HID], f32r, kind="ExternalInput")
    wo_d = nc.dram_tensor("wo", [HID, DQ], f32r, kind="ExternalInput")
    bq_d = nc.dram_tensor("bq", [P, N_PAIR], f32, kind="ExternalInput")
    bv_d = nc.dram_tensor("bv", [HD, NH], f32, kind="ExternalInput")
    gam_d = nc.dram_tensor("gamma", [1, DQ], f32r, kind="ExternalInput")
    bet_d = nc.dram_tensor("beta", [1, DQ], f32r, kind="ExternalInput")
    out_d = nc.dram_tensor("out", [RQ, DQ], f32, kind="ExternalOutput")

    with tile.TileContext(nc) as tc:
        const_cm = tc.tile_pool(name="const", bufs=1)
        const = const_cm.__enter__()
        wq0 = const.tile([P, DPO, P], f32r)
        wq_r = wq_d.ap().rearrange("(po p) h -> p po h", p=P)
        nc.sync.dma_start(wq0[:], wq_r[:, :, 0:P])
        bq_all = const.tile([P, N_PAIR], f32)
        bv_all = const.tile([HD, NH], f32)
        nc.sync.dma_start(bq_all[:], bq_d.ap())
        nc.sync.dma_start(bv_all[:], bv_d.ap())
        qT_sb = const.tile([P, DPO, RQ], f32r)
        for po in range(DPO):
            nc.sync.dma_start(
                qT_sb[:, po], qT_d.ap().rearrange("(po p) q -> po p q", p=P)[po])
        eps_t = const.tile([P, 1], f32)
        nc.vector.memset(eps_t[:], EPS)
        gb_bc = const.tile([P, 2, DQ], f32)
        ctxT_sb = const.tile([P, N_PAIR, RQ], f32r)
        wo_sb = const.tile([P, DPO, DQ], f32r)

        kt_r = kt_d.ap().rearrange("(hp p) q -> hp p q", p=P)
        # va viewed [kvpo, p, quartet, 4*VA]
        va_r = va_d.ap().rearrange("(po p) (qt v) -> po p qt v", p=P, v=4 * VA)
        wo_r = wo_d.ap().rearrange("(po p) e -> po p e", p=P)

        with (
            tc.tile_pool(name="vpool", bufs=3) as vpool,
            tc.tile_pool(name="ktpool", bufs=3) as ktpool,
            tc.tile_pool(name="qtpool", bufs=3) as qtpool,
            tc.tile_pool(name="epool", bufs=5) as epool,
            tc.tile_pool(name="wpool", bufs=2) as wpool,
            tc.tile_pool(name="bpool", bufs=3) as bpool,
            tc.tile_pool(name="smpool", bufs=4) as smpool,
            tc.tile_pool(name="sc_ps", bufs=2, space="PSUM") as sc_ps,
            tc.tile_pool(name="ctx_ps", bufs=4, space="PSUM") as ctx_ps,
        ):
            def q_proj(hp, name):
                if hp == 0:
                    wq_blk = wq0
                else:
                    wq_blk = wpool.tile([P, DPO, P], f32r, tag="w",
                                        name=f"wqb{hp}")
                    nc.sync.dma_start(wq_blk[:],
                                      wq_r[:, :, P * hp:P * (hp + 1)])
                ps_q = ctx_ps.tile([P, RQ], f32, tag="ctx", name=f"psq{hp}")
                for po in range(DPO):
                    nc.tensor.matmul(ps_q[:], wq_blk[:, po], qT_sb[:, po],
                                     start=(po == 0), stop=(po == DPO - 1))
                qt_t = qtpool.tile([P, RQ], f32r, tag="qt", name=name)
                nc.vector.tensor_scalar(qt_t[:], ps_q[:],
                                        bq_all[:, hp:hp + 1], None, op0=ALU.add)
                return qt_t

            qt_next = None
            for hp in range(N_PAIR):
                # K^T for pair straight from DRAM
                kt_pair = ktpool.tile([P, LKV], f32r, tag="kt")
                for kc in range(4):
                    nc.sync.dma_start(kt_pair[:, 512 * kc:512 * (kc + 1)],
                                      kt_r[hp, :, 512 * kc:512 * (kc + 1)])
                nc.sync.dma_start(wo_sb[:, hp], wo_r[hp])
                if hp == 0:
                    for i, rd in enumerate((gam_d, bet_d)):
                        row = bpool.tile([1, DQ], f32r, tag="recbc",
                                         name=f"row{i}")
                        nc.sync.dma_start(row[:], rd.ap())
                        nc.gpsimd.partition_broadcast(gb_bc[:, i, :],
                                                      row[:].bitcast(f32))
                # V quartet from DRAM
                if hp % 2 == 0:
                    qt4 = hp // 2
                    v_sb = vpool.tile([P, KV_T, 4 * VA], f32r, tag="v")
                    nc.sync.dma_start(
                        v_sb[:],
                        va_r[:, :, qt4, :].rearrange("po p v -> p po v"))

                # Q^T projection (pair 0 inline; later pairs were hoisted)
                qt_pair = q_proj(0, "qt0") if hp == 0 else qt_next

                # attention
                ps_c = [ctx_ps.tile([VA, RQ], f32, tag="ctx", name=f"ps_c{_h}")
                        for _h in range(2)]
                for kv in range(KV_T):
                    ps_s = sc_ps.tile([P, 2, RQ], f32, tag="sc")
                    for h in range(2):
                        lo, hi = HD * h, HD * (h + 1)
                        nc.tensor.matmul(
                            ps_s[:, h], kt_pair[lo:hi, P * kv:P * (kv + 1)],
                            qt_pair[lo:hi, :], start=True, stop=True,
                            tile_position=(HD * h, 0))
                    e_t = epool.tile([P, 2, RQ], f32r, tag="e")
                    nc.scalar.activation(e_t[:], ps_s[:], AF.Exp,
                                         scale=1.0 / np.sqrt(HD))
                    for h in range(2):
                        hq = (hp % 2) * 2 + h
                        nc.tensor.matmul(
                            ps_c[h][:],
                            v_sb[:, kv, VA * hq:VA * (hq + 1)],
                            e_t[:, h], start=(kv == 0), stop=(kv == KV_T - 1))

                if hp < N_PAIR - 1:
                    qt_next = q_proj(hp + 1, f"qt{hp + 1}")

                # normalize + bv
                for h in range(2):
                    rec = smpool.tile([1, RQ], f32, tag="rec")
                    nc.vector.reciprocal(rec[:], ps_c[h][HD:HD + 1, :])
                    rec_bc = bpool.tile([HD, RQ], f32, tag="recbc")
                    nc.gpsimd.partition_broadcast(rec_bc[:], rec[:])
                    dst = ctxT_sb[HD * h:HD * (h + 1), hp, :]
                    nc.vector.tensor_tensor(dst, ps_c[h][:HD, :], rec_bc[:],
                                            op=ALU.mult)
                    nc.vector.tensor_scalar(
                        dst, dst, bv_all[:, 2 * hp + h:2 * hp + h + 1], None,
                        op0=ALU.add)

        # output projection + residual + LayerNorm
        with (
            tc.tile_pool(name="opool", bufs=2) as opool,
            tc.tile_pool(name="xqpool", bufs=4) as xqpool,
            tc.tile_pool(name="ln_sm", bufs=4) as ln_sm,
            tc.tile_pool(name="out_ps", bufs=4, space="PSUM") as out_ps,
        ):
            ps_os = [out_ps.tile([P, 2, 512], f32, tag="o", name=f"pso{_m}")
                     for _m in range(MQ)]
            xq_ts = []
            for m in range(MQ):
                xq_t = xqpool.tile([P, DQ], f32, tag="xq", name=f"xq{m}")
                nc.sync.dma_start(
                    xq_t[:], xq_d.ap().rearrange("(m p) e -> m p e", p=P)[m])
                xq_ts.append(xq_t)
            for m in range(MQ):
                for po in range(DPO):
                    for n in range(2):
                        nc.tensor.matmul(
                            ps_os[m][:, n], ctxT_sb[:, po, P * m:P * (m + 1)],
                            wo_sb[:, po, 512 * n:512 * (n + 1)],
                            start=(po == 0), stop=(po == DPO - 1))
                xq_t = xq_ts[m]
                x = opool.tile([P, DQ], f32, tag="x")
                mu = ln_sm.tile([P, 1], f32, tag="mu")
                nc.vector.scalar_tensor_tensor(
                    x[:], ps_os[m][:].rearrange("p a b -> p (a b)"), 1.0,
                    xq_t[:], op0=ALU.mult, op1=ALU.add, accum_out=mu[:])
                xx = opool.tile([P, DQ], f32, tag="xx")
                m2 = ln_sm.tile([P, 1], f32, tag="m2")
                nc.scalar.activation(xx[:], x[:], AF.Square, accum_out=m2[:])
                nc.vector.tensor_scalar(mu[:], mu[:], 1.0 / DQ, None,
                                        op0=ALU.mult)
                musq = ln_sm.tile([P, 1], f32, tag="musq")
                nc.vector.tensor_tensor(musq[:], mu[:], mu[:], op=ALU.mult)
                var = ln_sm.tile([P, 1], f32, tag="var")
                nc.vector.tensor_scalar(var[:], m2[:], 1.0 / DQ, None,
                                        op0=ALU.mult)
                nc.vector.tensor_tensor(var[:], var[:], musq[:],
                                        op=ALU.subtract)
                sd = ln_sm.tile([P, 1], f32, tag="sd")
                nc.scalar.activation(sd[:], var[:], AF.Sqrt, bias=eps_t[:])
                rstd = ln_sm.tile([P, 1], f32, tag="rstd")
                nc.vector.reciprocal(rstd[:], sd[:])
                y = opool.tile([P, DQ], f32, tag="xx")
                nc.vector.scalar_tensor_tensor(
                    y[:], x[:], mu[:], gb_bc[:, 0], op0=ALU.subtract,
                    op1=ALU.mult)
                z = opool.tile([P, DQ], f32, tag="x")
                nc.vector.tensor_scalar(z[:], y[:], rstd[:], None, op0=ALU.mult)
                z2 = opool.tile([P, DQ], f32, tag="xx")
                nc.gpsimd.tensor_tensor(z2[:], z[:], gb_bc[:, 1], op=ALU.add)
                nc.sync.dma_start(
                    out_d.ap().rearrange("(m p) e -> m p e", p=P)[m], z2[:])
        const_cm.__exit__(None, None, None)

    nc.compile()
    return nc


_CACHE = {}


def _get(name):
    if name not in _CACHE:
        _CACHE[name] = build_phase1() if name == "p1" else build_phase2()
    return _CACHE[name]


def kernel(query, key_value, Wq, bq, Wk, bk, Wv, bv, Wo, bo, ln_gamma, ln_beta):
    query = np.asarray(query, dtype=np.float32)
    key_value = np.asarray(key_value, dtype=np.float32)
    Wq = np.ascontiguousarray(np.asarray(Wq, np.float32))
    Wk = np.ascontiguousarray(np.asarray(Wk, np.float32))
    Wv = np.ascontiguousarray(np.asarray(Wv, np.float32))
    Wo = np.ascontiguousarray(np.asarray(Wo, np.float32))
    bq_a = np.ascontiguousarray(np.asarray(bq, np.float32).reshape(N_PAIR, P).T)
    bk_a = np.ascontiguousarray(np.asarray(bk, np.float32).reshape(DPO, P).T)
    bv_a = np.ascontiguousarray(np.asarray(bv, np.float32).reshape(NH, HD).T)
    gam = np.asarray(ln_gamma, np.float32).reshape(1, DQ)
    bet = np.asarray(ln_beta, np.float32).reshape(1, DQ)
    bo = np.asarray(bo, np.float32)

    # ---- phase 1: K^T / V projections, kv-sharded ----
    nc1 = _get("p1")
    kvT = [np.ascontiguousarray(key_value[b].T) for b in range(B)]
    in1 = []
    for c in range(N_CORES):
        b, rk = divmod(c, N_CORES // B)
        cols = slice(RKV * rk, RKV * (rk + 1))
        in1.append({
            "kvTs": np.ascontiguousarray(kvT[b][:, cols]),
            "wk": Wk, "wv": Wv, "bk": bk_a,
        })
    run_bass_kernel_spmd(nc1, in1, list(range(N_CORES)))
    r1 = run_bass_kernel_spmd(nc1, in1, list(range(N_CORES))).results

    kt_full = [np.concatenate([r1[4 * b + i]["ktp"] for i in range(4)], axis=1)
               for b in range(B)]
    v_full = [np.concatenate([r1[4 * b + i]["vp"] for i in range(4)], axis=0)
              for b in range(B)]
    va_full = []
    for b in range(B):
        va = np.ones((LKV, NH, VA), np.float32)
        va[:, :, :HD] = v_full[b].reshape(LKV, NH, HD)
        va_full.append(va.reshape(LKV, NH * VA))

    # ---- phase 2: attention ----
    nc2 = _get("p2")
    in2 = []
    for c in range(N_CORES):
        b, rq = divmod(c, N_CORES // B)
        rows = slice(RQ * rq, RQ * (rq + 1))
        in2.append({
            "qT": np.ascontiguousarray(query[b, rows].T),
            "kt": kt_full[b], "va": va_full[b],
            "xq": np.ascontiguousarray(query[b, rows] + bo),
            "wq": Wq, "wo": Wo, "bq": bq_a, "bv": bv_a,
            "gamma": gam, "beta": bet,
        })
    run_bass_kernel_spmd(nc2, in2, list(range(N_CORES)))
    res = run_bass_kernel_spmd(nc2, in2, list(range(N_CORES)))
    out = np.concatenate([r["out"] for r in res.results], axis=0)
    return out.reshape(B, LQ, DQ)



# revision 22
# speedup vs baseline: 1.5320x; 1.5320x over previous
"""Cross-attention layer (B=2, L=2048, D=1024, 16 heads) on 8 TRN2 NeuronCores.

fp8e4m3 + DoubleRow rewrite. Two-phase pipeline:

Phase 1 (kv-sharded, core c -> batch c//4, kv rows 512*(c%4)..):
    K_s^T[hid', kv] = (32*Wk')^T kvT + 32*bk'   (fp8 out, hid' = scores layout)
    V_s[kv, hid]    = kv @ (32*Wv)              (fp8 out)
All matmuls are fp8 DoubleRow (2 contraction chunks of 128 per instr,
0.5 cycles/row). Host regathers per batch and assembles va (V + 64
replicated 0.5-columns that produce the softmax denominator on psum
partitions 64..128 for free).

Phase 2 (q-row-sharded): Q-projection (DoubleRow), scores per head with
contraction 64 = 2x32 at PE tile row 32*(h%4), softmax exp split between
ACT (native Exp -> fp8) and DVE (Schraudolph int8 bit-trick -> fp8),
ctx via kv-chunk-paired DoubleRow, per-head renorm = DVE reciprocal of
the replicated den rows + DVE mult, output projection head-paired
DoubleRow, LayerNorm with Pool offload.

hid' permutation: hid'(g,t,s,hh) = head(4g+s), d(32t+hh) so Q-proj psum
chunks land directly in the scores stationary/moving layout.
"""

import numpy as np
import ml_dtypes

import concourse.mybir as mybir
import concourse.tile as tile
from concourse import bacc
from concourse.bass_utils import run_bass_kernel_spmd

dt = mybir.dt
AF = mybir.ActivationFunctionType
ALU = mybir.AluOpType
PM = mybir.MatmulPerfMode
F8 = ml_dtypes.float8_e4m3

P = 128
B, LQ, LKV = 2, 2048, 2048
DQ, DKV, HID, NH = 1024, 1024, 1024, 16
HD = HID // NH
EPS = 1e-5
N_CORES = 8
RQ = LQ * B // N_CORES             # 512 q rows per phase-2 core
RKV = LKV * B // N_CORES           # 512 kv rows per phase-1 core
WS = 32.0                          # weight scale (fp8 subnormal dodge)
ALPHA = 1.0 / (WS * WS * np.sqrt(HD))          # exp psum scale
SCH_A = 11.541560327111707                     # 8/ln(2): fp8e4m3 bits/nat
SCH_B = 55.54                                  # schraudolph bias (round-nearest)
DVE_PAIRS = (1, 3, 6)              # kv-pairs whose exp runs on DVE (rest ACT)
DVE_PAIRS_B = (1, 3, 5, 7)         # heavier DVE share on every 3rd head
VA = HD + 1                        # V columns + 1 den column
VW = 80                            # padded va width (16B pair-stride rule)


def build_phase1():
    nc = bacc.Bacc("TRN2", target_bir_lowering=False, debug=False,
                   num_devices=N_CORES)
    f32, f8 = dt.float32, dt.float8e4
    kvTs_d = nc.dram_tensor("kvTs", [DKV, RKV], f8, kind="ExternalInput")
    wk_d = nc.dram_tensor("wk", [DKV, HID], f8, kind="ExternalInput")
    wv_d = nc.dram_tensor("wv", [DKV, HID], f8, kind="ExternalInput")
    bk_d = nc.dram_tensor("bk", [P, 8], f32, kind="ExternalInput")
    ktp_d = nc.dram_tensor("ktp", [HID, RKV], f8, kind="ExternalOutput")
    vp_d = nc.dram_tensor("vp", [RKV, HID], f8, kind="ExternalOutput")

    with tile.TileContext(nc) as tc:
        with (
            tc.tile_pool(name="c1", bufs=1) as c1,
            tc.tile_pool(name="op", bufs=2) as op,
            tc.tile_pool(name="ps", bufs=4, space="PSUM") as ps,
        ):
            kvTs = c1.tile([P, 4, 2, RKV], f8)
            nc.sync.dma_start(
                kvTs[:], kvTs_d.ap().rearrange("(j i p) k -> p j i k",
                                               p=P, i=2))
            bk_sb = c1.tile([P, 8], f32)
            nc.sync.dma_start(bk_sb[:], bk_d.ap())
            wk_sb = c1.tile([P, 4, 2, HID], f8)
            wk_r = wk_d.ap().rearrange("(j i p) (b h) -> b p j i h",
                                       p=P, i=2, h=512)
            for bh in range(2):
                nc.sync.dma_start(wk_sb[:, :, :, 512 * bh:512 * (bh + 1)],
                                  wk_r[bh])
            wv_sb = c1.tile([P, 4, 2, HID], f8)
            nc.sync.dma_start(
                wv_sb[:], wv_d.ap().rearrange("(j i p) h -> p j i h",
                                              p=P, i=2))

            # K chunks: cast into 4-chunk buffers, one out-DMA per buffer
            for half in range(2):
                kt_buf = op.tile([P, 4, RKV], f8, tag="kt", name=f"ktb{half}")
                for ci in range(4):
                    c = 4 * half + ci
                    ps_k = ps.tile([P, RKV], f32, tag="ps", name=f"k{c}")
                    for j in range(4):
                        nc.tensor.matmul(
                            ps_k[:], wk_sb[:, j, :, P * c:P * (c + 1)],
                            kvTs[:, j], start=(j == 0), stop=(j == 3),
                            perf_mode=PM.DoubleRow)
                    nc.scalar.activation(kt_buf[:, ci, :], ps_k[:],
                                         AF.Identity, bias=bk_sb[:, c:c + 1])
                nc.sync.dma_start(
                    ktp_d.ap().rearrange("(half c p) k -> half p c k",
                                         p=P, c=4)[half], kt_buf[:])

            # V: cast into per-half buffers, one out-DMA per hid half
            for n in range(2):
                v_buf = op.tile([P, 4, 512], f8, tag="v", name=f"vb{n}")
                for t in range(4):
                    ps_v = ps.tile([P, 512], f32, tag="ps", name=f"v{t}{n}")
                    for j in range(4):
                        nc.tensor.matmul(
                            ps_v[:], kvTs[:, j, :, P * t:P * (t + 1)],
                            wv_sb[:, j, :, 512 * n:512 * (n + 1)],
                            start=(j == 0), stop=(j == 3),
                            perf_mode=PM.DoubleRow)
                    nc.vector.tensor_copy(v_buf[:, t, :], ps_v[:])
                nc.sync.dma_start(
                    vp_d.ap().rearrange("(t p) (n f) -> n p t f",
                                        p=P, f=512)[n], v_buf[:])
    nc.compile()
    return nc


def build_phase2():
    nc = bacc.Bacc("TRN2", target_bir_lowering=False, debug=False,
                   num_devices=N_CORES)
    f32, f8 = dt.float32, dt.float8e4
    qT_d = nc.dram_tensor("qT", [DQ, RQ], f8, kind="ExternalInput")
    kt_d = nc.dram_tensor("kt", [HID, LKV], f8, kind="ExternalInput")
    va_d = nc.dram_tensor("va", [LKV, NH * VW], f8, kind="ExternalInput")
    xq_d = nc.dram_tensor("xq", [RQ, DQ], f32, kind="ExternalInput")
    wq_d = nc.dram_tensor("wq", [DQ, HID], f8, kind="ExternalInput")
    wo_d = nc.dram_tensor("wo", [HD, NH * DQ], f8, kind="ExternalInput")
    bq_d = nc.dram_tensor("bq", [P, 8], f32, kind="ExternalInput")
    gam_d = nc.dram_tensor("gamma", [1, DQ], f32, kind="ExternalInput")
    bet_d = nc.dram_tensor("beta", [1, DQ], f32, kind="ExternalInput")
    out_d = nc.dram_tensor("out", [RQ, DQ], f32, kind="ExternalOutput")

    with tile.TileContext(nc) as tc:
        const_cm = tc.tile_pool(name="const", bufs=1)
        const = const_cm.__enter__()
        # DMA order matters: the attention pipeline needs qT/wq/kt first;
        # va streams per kv-pair behind it; wo/xq only matter at the end.
        qT_sb = const.tile([P, 4, 2, RQ], f8)
        nc.sync.dma_start(
            qT_sb[:], qT_d.ap().rearrange("(j i p) q -> p j i q", p=P, i=2))
        wq_sb = const.tile([P, 4, 2, HID], f8)
        nc.sync.dma_start(
            wq_sb[:], wq_d.ap().rearrange("(j i p) h -> p j i h", p=P, i=2))
        kt_sb = const.tile([P, 4, 2, LKV], f8)
        kt_r = kt_d.ap().rearrange("(g t p) k -> g p t k", p=P, t=2)
        nc.sync.dma_start(kt_sb[:, 0], kt_r[0])
        bq_sb = const.tile([P, 8], f32)
        nc.sync.dma_start(bq_sb[:], bq_d.ap())
        for g in range(1, 4):
            nc.sync.dma_start(kt_sb[:, g], kt_r[g])
        va_sb = const.tile([P, 8, 2, NH, VW], f8)
        va_r = va_d.ap().rearrange("(j i p) (h v) -> j p i h v", p=P, i=2,
                                   v=VW)
        for j in range(8):
            nc.sync.dma_start(va_sb[:, j], va_r[j])
        wo_sb = const.tile([HD, NH, DQ], f8)
        nc.sync.dma_start(
            wo_sb[:], wo_d.ap().rearrange("p (h e) -> p h e", e=DQ))
        xq_sb = const.tile([P, 4, DQ], f32)
        nc.sync.dma_start(
            xq_sb[:], xq_d.ap().rearrange("(m p) e -> p m e", p=P))
        eps_t = const.tile([P, 1], f32)
        nc.vector.memset(eps_t[:], EPS)
        gb_bc = const.tile([P, 2, DQ], f32)
        qt_sb = const.tile([P, 4, 2, RQ], f8)
        ctxT_sb = const.tile([HD, NH, RQ], f8)

        with (
            tc.tile_pool(name="epool", bufs=2) as epool,
            tc.tile_pool(name="rpool", bufs=2) as rpool,
            tc.tile_pool(name="bpool", bufs=2) as bpool,
            tc.tile_pool(name="sp_ps", bufs=3, space="PSUM") as sp_ps,
            tc.tile_pool(name="ctx_ps", bufs=2, space="PSUM") as ctx_ps,
        ):
            def q_proj(g):
                for t in range(2):
                    c = 2 * g + t
                    ps_q = sp_ps.tile([P, RQ], f32, tag="sp", name=f"psq{c}")
                    for j in range(4):
                        nc.tensor.matmul(
                            ps_q[:], wq_sb[:, j, :, P * c:P * (c + 1)],
                            qT_sb[:, j], start=(j == 0), stop=(j == 3),
                            perf_mode=PM.DoubleRow)
                    nc.scalar.activation(qt_sb[:, g, t, :], ps_q[:],
                                         AF.Identity, bias=bq_sb[:, c:c + 1])

            q_proj(0)
            for i, rd in enumerate((gam_d, bet_d)):
                row = bpool.tile([1, DQ], f32, tag="gb", name=f"row{i}")
                nc.sync.dma_start(row[:], rd.ap())
                nc.gpsimd.partition_broadcast(gb_bc[:, i, :], row[:])

            for g in range(4):
                for s in range(4):
                    h = 4 * g + s
                    lo = 32 * s
                    e_t = epool.tile([P, 16, RQ], f8, tag="e", name=f"e{h}")
                    ps_c = ctx_ps.tile([P, RQ], f32, tag="ctx",
                                       name=f"ctx{h}")

                    def ctx_mm(j):
                        nc.tensor.matmul(
                            ps_c[:VA], va_sb[:, j, :, h, :VA],
                            e_t[:, 2 * j:2 * j + 2, :],
                            start=(j == 0), stop=(j == 7),
                            perf_mode=PM.DoubleRow)

                    for k in range(8):
                        ps_s = sp_ps.tile([P, 2, RQ], f32, tag="sp",
                                          name=f"s{h}_{k}")
                        for i in range(2):
                            ch = 2 * k + i
                            nc.tensor.matmul(
                                ps_s[:, i],
                                kt_sb[lo:lo + 32, g, :, P * ch:P * (ch + 1)],
                                qt_sb[lo:lo + 32, g, :, :],
                                start=True, stop=True,
                                perf_mode=PM.DoubleRow,
                                tile_position=(lo, 0))
                        # whole-pair exp, alternating engines (~ACT 4.6 : DVE 3.4)
                        if k in (DVE_PAIRS_B if h % 3 == 2 else DVE_PAIRS):
                            nc.vector.tensor_scalar(
                                e_t[:, 2 * k:2 * k + 2, :].bitcast(dt.int8),
                                ps_s[:], SCH_A * ALPHA, SCH_B,
                                op0=ALU.mult, op1=ALU.add)
                        else:
                            nc.scalar.activation(e_t[:, 2 * k:2 * k + 2, :],
                                                 ps_s[:], AF.Exp, scale=ALPHA)
                        # ctx lags one pair so PE never waits on exp
                        if k > 0:
                            ctx_mm(k - 1)
                    ctx_mm(7)
                    rec = rpool.tile([VA, RQ], f32, tag="r", name=f"rec{h}")
                    nc.vector.reciprocal(rec[HD:VA, :], ps_c[HD:VA, :])
                    rec_bc = bpool.tile([HD, RQ], f32, tag="rb",
                                        name=f"rb{h}")
                    nc.gpsimd.partition_broadcast(rec_bc[:], rec[HD:VA, :])
                    nc.vector.tensor_tensor(ctxT_sb[:, h, :], ps_c[:HD, :],
                                            rec_bc[:], op=ALU.mult)
                    if s == 2 and g < 3:
                        q_proj(g + 1)

        # output projection + residual + LayerNorm
        with (
            tc.tile_pool(name="opool", bufs=2) as opool,
            tc.tile_pool(name="zpool", bufs=4) as zpool,
            tc.tile_pool(name="ln_sm", bufs=4) as ln_sm,
            tc.tile_pool(name="out_ps", bufs=4, space="PSUM") as out_ps,
        ):
            for m in range(4):
                ps_o = out_ps.tile([P, 2, 512], f32, tag="o", name=f"o{m}")
                for n in range(2):
                    for u in range(8):
                        nc.tensor.matmul(
                            ps_o[:, n],
                            ctxT_sb[:, 2 * u:2 * u + 2, P * m:P * (m + 1)],
                            wo_sb[:, 2 * u:2 * u + 2, 512 * n:512 * (n + 1)],
                            start=(u == 0), stop=(u == 7),
                            perf_mode=PM.DoubleRow)
                x = opool.tile([P, DQ], f32, tag="x", name=f"x{m}")
                mu = ln_sm.tile([P, 1], f32, tag="mu")
                nc.vector.scalar_tensor_tensor(
                    x[:], ps_o[:].rearrange("p a b -> p (a b)"),
                    1.0 / (WS * WS * 2.0), xq_sb[:, m], op0=ALU.mult,
                    op1=ALU.add, accum_out=mu[:])
                xx = opool.tile([P, DQ], f32, tag="xx", name=f"xx{m}")
                m2 = ln_sm.tile([P, 1], f32, tag="m2")
                nc.scalar.activation(xx[:], x[:], AF.Square, accum_out=m2[:])
                nc.vector.tensor_scalar(mu[:], mu[:], 1.0 / DQ, None,
                                        op0=ALU.mult)
                musq = ln_sm.tile([P, 1], f32, tag="musq")
                nc.vector.tensor_tensor(musq[:], mu[:], mu[:], op=ALU.mult)
                var = ln_sm.tile([P, 1], f32, tag="var")
                nc.vector.tensor_scalar(var[:], m2[:], 1.0 / DQ, None,
                                        op0=ALU.mult)
                nc.vector.tensor_tensor(var[:], var[:], musq[:],
                                        op=ALU.subtract)
                sd = ln_sm.tile([P, 1], f32, tag="sd")
                nc.scalar.activation(sd[:], var[:], AF.Sqrt, bias=eps_t[:])
                rstd = ln_sm.tile([P, 1], f32, tag="rstd")
                nc.vector.reciprocal(rstd[:], sd[:])
                musr = ln_sm.tile([P, 1], f32, tag="musr")
                nc.vector.tensor_tensor(musr[:], mu[:], rstd[:], op=ALU.mult)
                t1 = zpool.tile([P, DQ], f32, tag="t1", name=f"t1{m}")
                nc.scalar.activation(t1[:], x[:], AF.Copy, scale=rstd[:])
                z = zpool.tile([P, DQ], f32, tag="z", name=f"z{m}")
                z_eng = nc.gpsimd if m % 2 == 0 else nc.vector
                nc.vector.scalar_tensor_tensor(
                    z[:], t1[:], musr[:], gb_bc[:, 0], op0=ALU.subtract,
                    op1=ALU.mult)
                z2 = zpool.tile([P, DQ], f32, tag="z2", name=f"z2{m}")
                z_eng.tensor_tensor(z2[:], z[:], gb_bc[:, 1], op=ALU.add)
                nc.sync.dma_start(
                    out_d.ap().rearrange("(m p) e -> m p e", p=P)[m], z2[:])
        const_cm.__exit__(None, None, None)

    nc.compile()
    return nc


_CACHE = {}


def _get(name):
    if name not in _CACHE:
        _CACHE[name] = build_phase1() if name == "p1" else build_phase2()
    return _CACHE[name]


def _hid_perm():
    """hid'(g,t,s,hh) -> original hid index (head=4g+s, d=32t+hh)."""
    perm = np.empty(HID, np.int64)
    i = 0
    for g in range(4):
        for t in range(2):
            for s in range(4):
                for hh in range(32):
                    perm[i] = (4 * g + s) * HD + 32 * t + hh
                    i += 1
    return perm


def kernel(query, key_value, Wq, bq, Wk, bk, Wv, bv, Wo, bo, ln_gamma,
           ln_beta):
    query = np.asarray(query, dtype=np.float32)
    key_value = np.asarray(key_value, dtype=np.float32)
    Wq = np.asarray(Wq, np.float32)
    Wk = np.asarray(Wk, np.float32)
    Wv = np.asarray(Wv, np.float32)
    Wo = np.asarray(Wo, np.float32)
    bq = np.asarray(bq, np.float32)
    bk = np.asarray(bk, np.float32)
    bv = np.asarray(bv, np.float32)
    bo = np.asarray(bo, np.float32)
    gam = np.asarray(ln_gamma, np.float32).reshape(1, DQ)
    bet = np.asarray(ln_beta, np.float32).reshape(1, DQ)

    perm = _hid_perm()
    wq8 = np.ascontiguousarray(WS * Wq[:, perm]).astype(F8)
    wk8 = np.ascontiguousarray(WS * Wk[:, perm]).astype(F8)
    wv8 = np.ascontiguousarray(WS * Wv).astype(F8)
    # wo rows reordered to [hd, head, e]
    wo8 = np.ascontiguousarray(
        (WS * Wo).reshape(NH, HD, DQ).transpose(1, 0, 2).reshape(HD, NH * DQ)
    ).astype(F8)
    bq_a = np.ascontiguousarray((WS * bq[perm]).reshape(8, P).T)
    bk_a = np.ascontiguousarray((WS * bk[perm]).reshape(8, P).T)
    xq_base = (bo + bv @ Wo).astype(np.float32)

    # ---- phase 1: K^T / V projections, kv-sharded ----
    nc1 = _get("p1")
    in1 = []
    for c in range(N_CORES):
        b, rk = divmod(c, N_CORES // B)
        cols = slice(RKV * rk, RKV * (rk + 1))
        kvT = np.ascontiguousarray(key_value[b].T[:, cols]).astype(F8)
        in1.append({"kvTs": kvT, "wk": wk8, "wv": wv8, "bk": bk_a})
    run_bass_kernel_spmd(nc1, in1, list(range(N_CORES)))
    r1 = run_bass_kernel_spmd(nc1, in1, list(range(N_CORES))).results

    kt_full = [np.concatenate([r1[4 * b + i]["ktp"] for i in range(4)],
                              axis=1) for b in range(B)]
    va_full = []
    for b in range(B):
        vp = np.concatenate([r1[4 * b + i]["vp"] for i in range(4)], axis=0)
        va = np.zeros((LKV, NH, VW), F8)
        va[:, :, :HD] = vp.reshape(LKV, NH, HD)
        va[:, :, HD] = F8(0.5)
        va_full.append(va.reshape(LKV, NH * VW))

    # ---- phase 2: attention ----
    nc2 = _get("p2")
    in2 = []
    for c in range(N_CORES):
        b, rq = divmod(c, N_CORES // B)
        rows = slice(RQ * rq, RQ * (rq + 1))
        in2.append({
            "qT": np.ascontiguousarray(query[b, rows].T).astype(F8),
            "kt": kt_full[b], "va": va_full[b],
            "xq": np.ascontiguousarray(query[b, rows] + xq_base),
            "wq": wq8, "wo": wo8, "bq": bq_a,
            "gamma": gam, "beta": bet,
        })
    run_bass_kernel_spmd(nc2, in2, list(range(N_CORES)))
    res = run_bass_kernel_spmd(nc2, in2, list(range(N_CORES)))
    out = np.concatenate([r["out"] for r in res.results], axis=0)
    return out.reshape(B, LQ, DQ)
